# revision 1
# baseline (speedup 1.0000x reference)
import sys
import numpy as np

for _p in ("/opt/trn_rl_repo",):
    if _p not in sys.path:
        sys.path.insert(0, _p)

import concourse.bass as bass
import concourse.mybir as mybir
import concourse.tile as tile
from concourse.bass_utils import run_bass_kernel_spmd

TEMPERATURE = 0.07
EPS = 1e-08
HARD_NEG_WEIGHT = 2.0
DOMAIN_SEP_WEIGHT = 1.5

B, L, D = 32, 256, 256
N = B * L            # 8192
NCORES = 8
ROWS = N // NCORES   # 1024 rows of sim per core
F32 = mybir.dt.float32

_CACHE = {}


def _build_nc():
    """Per-core program: simblk[1024, 8192] = xblkT.T @ xT (raw dot products)."""
    nc = bass.Bass()
    # xin = [xTfull | xTblk] concatenated along columns: one DMA -> one wait sem
    xin = nc.declare_dram_parameter("xin", [D, N + ROWS], F32, isOutput=False)
    simblk = nc.declare_dram_parameter("simblk", [ROWS, N], F32, isOutput=True)

    KT = D // 128          # 2 k-tiles
    MT = ROWS // 128       # 8 row tiles of 128 rows
    NBW = 512              # psum free dim
    NT = N // NBW          # 16 col tiles per row
    RB = 3                 # row staging buffers
    TILES = MT * NT        # 128 psum tiles total

    with (
        nc.sbuf_tensor([128, KT, N + ROWS], F32) as xf,
        nc.sbuf_tensor([128, RB, N], F32) as rows,
        nc.psum_tensor([128, 8, NBW], F32) as ps,
        nc.semaphore("s_in") as s_in,
        nc.semaphore("s_pe") as s_pe,
        nc.semaphore("s_cp") as s_cp,
        nc.semaphore("s_o0") as s_o0,
        nc.semaphore("s_o1") as s_o1,
        nc.semaphore("s_o2") as s_o2,
        nc.Block() as block,
    ):
        s_out = [s_o0, s_o1, s_o2]

        @block.sync
        def _(sync):
            sync.dma_start(
                xf[:], xin.rearrange("(k p) n -> p k n", p=128)
            ).then_inc(s_in, 16)
            for m in range(MT):
                # all 16 copies of row m staged
                sync.wait_ge(s_cp, 16 * (m + 1))
                sync.dma_start(
                    simblk[m * 128:(m + 1) * 128, :], rows[:, m % RB, :]
                ).then_inc(s_out[m % RB], 16)

        @block.tensor
        def _(tensor):
            tensor.wait_ge(s_in, 16)
            for t in range(TILES):
                m, n = divmod(t, NT)
                b = t % 8
                if t >= 8:
                    # psum bank b free once copy of tile t-8 completed
                    tensor.wait_ge(s_cp, t - 7)
                for k in range(KT):
                    nc.tensor.matmul(
                        ps[:, b, :],
                        xf[:, k, N + m * 128:N + (m + 1) * 128],
                        xf[:, k, n * NBW:(n + 1) * NBW],
                        start=(k == 0),
                        stop=(k == KT - 1),
                    ).then_inc(s_pe, 1)

        @block.vector
        def _(vector):
            for t in range(TILES):
                m, n = divmod(t, NT)
                if n == 0 and m >= RB:
                    # row slot reusable once row m-RB fully DMAed out
                    vector.wait_ge(s_out[m % RB], 16 * (m // RB))
                vector.wait_ge(s_pe, 2 * (t + 1))
                nc.vector.tensor_copy(
                    rows[:, m % RB, n * NBW:(n + 1) * NBW], ps[:, t % 8, :]
                ).then_inc(s_cp, 1)

    return nc


def _get_nc():
    if "nc" not in _CACHE:
        _CACHE["nc"] = _build_nc()
    return _CACHE["nc"]


def _run_device(xT, trace=False):
    nc = _get_nc()
    in_maps = [
        {
            "xin": np.ascontiguousarray(
                np.concatenate([xT, xT[:, c * ROWS:(c + 1) * ROWS]], axis=1)
            ),
        }
        for c in range(NCORES)
    ]
    try:
        res = run_bass_kernel_spmd(nc, in_maps, list(range(NCORES)), trace=trace)
    except ModuleNotFoundError:
        # NTFF profile hook unavailable in this container; run without trace
        res = run_bass_kernel_spmd(nc, in_maps, list(range(NCORES)), trace=False)
    sim = np.concatenate([res.results[c]["simblk"] for c in range(NCORES)], axis=0)
    return sim, res


def kernel(feats, dataset_ids, image_ids, _trace=False, _ret_res=False):
    x = np.asarray(feats, dtype=np.float32).reshape(N, D)
    nrm = np.sqrt(np.sum(x * x, axis=1, keepdims=True, dtype=np.float32)).astype(np.float32)
    x = x / np.maximum(nrm, np.float32(EPS))
    xT = np.ascontiguousarray(x.T)

    sim, res = _run_device(xT, trace=_trace)
    sim = sim / np.float32(TEMPERATURE)

    did = np.asarray(dataset_ids).reshape(-1)
    iid = np.asarray(image_ids).reshape(-1)
    same_img = (did[:, None] == did[None, :]) & (iid[:, None] == iid[None, :])
    eye = np.eye(N, dtype=bool)
    pos_mask = same_img & ~eye
    diff_dataset = did[:, None] != did[None, :]

    sim_exp = np.exp(sim)

    cross = sim[diff_dataset]
    if cross.size > 0:
        thr = np.float32(np.quantile(cross, 0.8))
    else:
        thr = np.float32(0.0)
    hard_neg_mask = diff_dataset & (sim > thr)

    neg_weights = np.where(diff_dataset, np.float32(DOMAIN_SEP_WEIGHT), np.float32(1.0)) * \
        np.where(hard_neg_mask, np.float32(HARD_NEG_WEIGHT), np.float32(1.0))

    pos_sum = np.sum(sim_exp * pos_mask.astype(np.float32), axis=1, dtype=np.float32)
    neg_sum = np.sum(sim_exp * neg_weights * (~pos_mask).astype(np.float32), axis=1, dtype=np.float32)

    loss = -np.log((pos_sum + np.float32(EPS)) / (pos_sum + neg_sum + np.float32(EPS)))

    valid = pos_mask.any(axis=1)
    n_valid = valid.sum()
    if n_valid > 0:
        out = np.float32(np.sum(loss * valid.astype(np.float32)) / np.float32(max(n_valid, 1)))
    else:
        out = np.float32(loss.mean())
    out = np.asarray(out, dtype=np.float32)
    if _ret_res:
        return out, res
    return out



# revision 4
# speedup vs baseline: 24.4040x; 24.4040x over previous
import sys
import numpy as np

for _p in ("/opt/trn_rl_repo",):
    if _p not in sys.path:
        sys.path.insert(0, _p)

import ml_dtypes
import concourse.bass as bass
import concourse.mybir as mybir
from concourse.bass_utils import run_bass_kernel_spmd

TEMPERATURE = 0.07
EPS = 1e-08
HARD_NEG_WEIGHT = 2.0
DOMAIN_SEP_WEIGHT = 1.5

B, L, D = 32, 256, 256
N = B * L            # 8192
NCORES = 8
ROWS = N // NCORES   # 1024 rows of sim per core
RT = ROWS // 128     # 8 row tiles per core
CT = 16              # col tiles (512 wide) over N
NT = RT * CT         # 128 sim tiles per core
F32 = mybir.dt.float32
BF16 = mybir.dt.bfloat16
INV_T = float(1.0 / np.float32(TEMPERATURE))

EBUF = 4             # e (exp) staging buffers
MBANKS = 6           # psum banks for the sim matmul
NCH = 8              # xin column chunks

_CACHE = {}


def _build_nc():
    """Per-core program (raw bass, explicit semaphores).

    For each of its 1024 rows i (row-tile r in 0..7, partition p) computes
      tot[i]   = sum_j exp(sim[i,j])                      (incl. diagonal)
      pos[i]   = sum_{j: key_j==key_i, j!=i} exp(sim)     (exact diagonal kill)
      sdiff[i] = sum_{j: did_j!=did_i} exp(sim)
      shard[i] = sum_{j: did_j!=did_i, exp(sim)>ethr} exp(sim)
    with sim[i,j] = (x_i . x_j)/TEMPERATURE.  x is bf16; PE does the matmul
    into psum, ACT does exp with fused row-sum accumulation, DVE does the
    masked sums via fused scalar_tensor_tensor ops.  Output out[p, q, r].
    """
    nc = bass.Bass()
    AF = mybir.ActivationFunctionType
    OP = mybir.AluOpType

    xin = nc.declare_dram_parameter("xin", [128, 2, N + ROWS], BF16, isOutput=False)
    # rows4: [0]=key_j, [1]=did_j, [2]=j (global col idx), [3]=ones
    rows4 = nc.declare_dram_parameter("rows4", [4, N], F32, isOutput=False)
    # scal: [:,0:8]=key_i, [:,8:16]=did_i, [:,16:24]=i, [:,24]=ethr, [:,25]=0
    scal = nc.declare_dram_parameter("scal", [128, 26], F32, isOutput=False)
    # selm[k, s*128+m] = 1.0 if k == s else 0.0  (broadcast selector weights)
    selm = nc.declare_dram_parameter("selm", [4, 3 * 128], F32, isOutput=False)
    out = nc.declare_dram_parameter("out", [128, 4, RT], F32, isOutput=True)

    from contextlib import ExitStack
    with ExitStack() as ctx:
        xf = ctx.enter_context(nc.sbuf_tensor([128, 2, N + ROWS], BF16))
        kb = ctx.enter_context(nc.sbuf_tensor([128, N], F32))
        db = ctx.enter_context(nc.sbuf_tensor([128, N], F32))
        ib = ctx.enter_context(nc.sbuf_tensor([128, N], F32))
        r4 = ctx.enter_context(nc.sbuf_tensor([4, N], F32))
        sel = ctx.enter_context(nc.sbuf_tensor([4, 3 * 128], F32))
        sc = ctx.enter_context(nc.sbuf_tensor([128, 26], F32))
        ee = ctx.enter_context(nc.sbuf_tensor([128, EBUF, 512], F32))
        junk = ctx.enter_context(nc.sbuf_tensor([128, 2, 4, 512], F32))
        acc_tot = ctx.enter_context(nc.sbuf_tensor([128, NT], F32))
        acc_pos = ctx.enter_context(nc.sbuf_tensor([128, NT], F32))
        acc_sdf = ctx.enter_context(nc.sbuf_tensor([128, NT], F32))
        acc_shd = ctx.enter_context(nc.sbuf_tensor([128, NT], F32))
        fin = ctx.enter_context(nc.sbuf_tensor([128, 4, RT], F32))
        ps = ctx.enter_context(nc.psum_tensor([128, 8, 512], F32))
        s_sm = ctx.enter_context(nc.semaphore("s_sm"))    # rows4+scal (2x16)
        s_blk = ctx.enter_context(nc.semaphore("s_blk"))  # xin block DMA (16)
        s_ch = ctx.enter_context(nc.semaphore("s_ch"))    # xin chunks (16 ea)
        s_peb = ctx.enter_context(nc.semaphore("s_peb"))  # bcast matmuls
        s_vx = ctx.enter_context(nc.semaphore("s_vx"))    # DVE op counter
        s_pe = ctx.enter_context(nc.semaphore("s_pe"))    # sim matmul tiles
        s_act = ctx.enter_context(nc.semaphore("s_act"))  # exp tiles
        s_out = ctx.enter_context(nc.semaphore("s_out"))  # output DMA
        block = ctx.enter_context(nc.Block())
        @block.sync
        def _(sync):
            sync.dma_start(r4[:], rows4[:]).then_inc(s_sm, 16)
            sync.dma_start(sc[:], scal[:]).then_inc(s_sm, 16)
            sync.dma_start(sel[:], selm[:]).then_inc(s_sm, 16)
            sync.dma_start(xf[:, :, N:], xin[:, :, N:]).then_inc(s_blk, 16)
            for ch in range(NCH):
                sl = slice(ch * 1024, (ch + 1) * 1024)
                sync.dma_start(xf[:, :, sl], xin[:, :, sl]).then_inc(s_ch, 16)
            # final output once the DVE reduces are done (NT tiles + 4 reduces)
            sync.wait_ge(s_vx, 48 + 4 * NT + 4)
            sync.dma_start(out[:], fin[:]).then_inc(s_out, 16)
            sync.wait_ge(s_out, 16)

        @block.tensor
        def _(tensor):
            # --- broadcast id rows to 128 partitions: ones^T @ row ---
            tensor.wait_ge(s_sm, 48)
            for j in range(3 * CT):
                src, t = divmod(j, CT)
                if j >= 2:
                    tensor.wait_ge(s_vx, j - 1)  # bank j%2 free
                nc.tensor.matmul(
                    ps[:, 6 + j % 2, :],
                    sel[:, src * 128:(src + 1) * 128],
                    r4[:, t * 512:(t + 1) * 512],
                    start=True, stop=True,
                ).then_inc(s_peb, 1)
            # --- sim tiles ---
            tensor.wait_ge(s_blk, 16)
            for tau in range(NT):
                r, t = divmod(tau, CT)
                cs = slice(t * 512, (t + 1) * 512)
                if tau == 0:
                    # chunk DMA completions across queues are unordered on
                    # one counting sem, so wait for all of them up front
                    tensor.wait_ge(s_ch, 16 * NCH)
                if tau >= MBANKS:
                    tensor.wait_ge(s_act, tau - MBANKS + 1)  # bank reuse
                bank = tau % MBANKS
                nc.tensor.matmul(
                    ps[:, bank, :],
                    xf[:, 0, N + r * 128:N + (r + 1) * 128],
                    xf[:, 0, cs],
                    start=True, stop=False,
                )
                nc.tensor.matmul(
                    ps[:, bank, :],
                    xf[:, 1, N + r * 128:N + (r + 1) * 128],
                    xf[:, 1, cs],
                    start=False, stop=True,
                ).then_inc(s_pe, 1)

        @block.scalar
        def _(scalar):
            for tau in range(NT):
                scalar.wait_ge(s_pe, tau + 1)
                if tau >= EBUF:
                    scalar.wait_ge(s_vx, 48 + 4 * (tau - EBUF + 1))  # e slot
                nc.scalar.activation(
                    ee[:, tau % EBUF, :], ps[:, tau % MBANKS, :],
                    AF.Exp, bias=sc[:, 25:26], scale=INV_T,
                    accum_out=acc_tot[:, tau:tau + 1],
                ).then_inc(s_act, 1)

        @block.vector
        def _(vector):
            # DVE ops complete asynchronously (per-op DRAIN), so every
            # cross-instruction hazard on this engine is guarded via s_vx,
            # which counts completed DVE ops: 48 broadcast copies, then
            # 4 ops per sim tile (ma, en, pj, hj), then 4 reduces.
            vector.wait_ge(s_sm, 48)
            dsts = (kb, db, ib)
            for j in range(3 * CT):
                src, t = divmod(j, CT)
                vector.wait_ge(s_peb, j + 1)
                nc.vector.tensor_copy(
                    dsts[src][:, t * 512:(t + 1) * 512], ps[:, 6 + j % 2, :]
                ).then_inc(s_vx, 1)
            for tau in range(NT):
                r, t = divmod(tau, CT)
                cs = slice(t * 512, (t + 1) * 512)
                par = tau % 2
                e = ee[:, tau % EBUF, :]
                base = 48 + 4 * tau
                vector.wait_ge(s_act, tau + 1)
                # ma: keyeq * e    (junk slot WAR/WAW vs tile tau-2)
                if tau >= 2:
                    vector.wait_ge(s_vx, base - 8 + 3)
                else:
                    vector.wait_ge(s_vx, t + 1)  # kb col tile copy done
                nc.vector.scalar_tensor_tensor(
                    junk[:, par, 0, :], kb[:, cs], sc[:, r:r + 1], e,
                    OP.is_equal, OP.mult,
                ).then_inc(s_vx, 1)
                # en: didne * e -> sdiff
                if tau >= 2:
                    vector.wait_ge(s_vx, base - 8 + 4)
                else:
                    vector.wait_ge(s_vx, CT + t + 1)  # db col tile copy done
                nc.vector.scalar_tensor_tensor(
                    junk[:, par, 2, :], db[:, cs], sc[:, 8 + r:9 + r], e,
                    OP.not_equal, OP.mult,
                    accum_out=acc_sdf[:, tau:tau + 1],
                ).then_inc(s_vx, 1)
                # pj: ma * (j != i) -> pos (diagonal killed pre-reduction)
                vector.wait_ge(s_vx, base + 1)
                nc.vector.scalar_tensor_tensor(
                    junk[:, par, 1, :], ib[:, cs], sc[:, 16 + r:17 + r],
                    junk[:, par, 0, :],
                    OP.not_equal, OP.mult,
                    accum_out=acc_pos[:, tau:tau + 1],
                ).then_inc(s_vx, 1)
                # hj: (e > ethr) * en -> shard
                vector.wait_ge(s_vx, base + 2)
                nc.vector.scalar_tensor_tensor(
                    junk[:, par, 3, :], e, sc[:, 24:25], junk[:, par, 2, :],
                    OP.is_gt, OP.mult,
                    accum_out=acc_shd[:, tau:tau + 1],
                ).then_inc(s_vx, 1)
            vector.wait_ge(s_act, NT)
            vector.wait_ge(s_vx, 48 + 4 * NT)
            for q, a in enumerate((acc_tot, acc_pos, acc_sdf, acc_shd)):
                nc.vector.tensor_reduce(
                    fin[:, q:q + 1, :],
                    a[:].rearrange("p (r t) -> p r t", t=CT),
                    axis=mybir.AxisListType.X, op=OP.add,
                ).then_inc(s_vx, 1)

    return nc


def _get_nc():
    if "nc" not in _CACHE:
        _CACHE["nc"] = _build_nc()
    return _CACHE["nc"]


def _prep_inputs(feats, dataset_ids, image_ids):
    x = np.asarray(feats, dtype=np.float32).reshape(N, D)
    nrm = np.sqrt(np.sum(x * x, axis=1, keepdims=True, dtype=np.float32)).astype(np.float32)
    x = x / np.maximum(nrm, np.float32(EPS))

    did = np.asarray(dataset_ids).reshape(-1).astype(np.int64)
    iid = np.asarray(image_ids).reshape(-1).astype(np.int64)
    key = did * 128 + iid

    # threshold: global 0.8-quantile of cross-dataset sims, from a strided
    # host-side sample (loss sensitivity to thr is tiny: the diagonal
    # exp(1/T) ~ 1.6e6 dominates neg_sum)
    ethr = 1.0
    if np.unique(did).size > 1:
        rs, cs_ = 64, 8
        while True:
            ridx = np.arange(0, N, rs)
            cidx = np.arange(0, N, cs_)
            s = (x[ridx] @ x[cidx].T) / np.float32(TEMPERATURE)
            m = did[ridx][:, None] != did[cidx][None, :]
            vals = s[m]
            if vals.size >= 1000 or (rs == 1 and cs_ == 1):
                break
            rs = max(1, rs // 8)
            cs_ = max(1, cs_ // 8)
        thr = float(np.quantile(vals, 0.8))
        ethr = float(np.exp(thr))

    xT2 = np.ascontiguousarray(
        x.T.reshape(2, 128, N).transpose(1, 0, 2)
    ).astype(ml_dtypes.bfloat16)

    rows4 = np.zeros((4, N), dtype=np.float32)
    rows4[0] = key.astype(np.float32)
    rows4[1] = did.astype(np.float32)
    rows4[2] = np.arange(N, dtype=np.float32)
    selm = np.zeros((4, 3 * 128), dtype=np.float32)
    for s_ in range(3):
        selm[s_, s_ * 128:(s_ + 1) * 128] = 1.0

    in_maps = []
    for c in range(NCORES):
        blk = slice(c * ROWS, (c + 1) * ROWS)
        xin = np.ascontiguousarray(
            np.concatenate([xT2, xT2[:, :, blk]], axis=2)
        )
        sc = np.zeros((128, 26), dtype=np.float32)
        sc[:, 0:8] = key[blk].reshape(RT, 128).T.astype(np.float32)
        sc[:, 8:16] = did[blk].reshape(RT, 128).T.astype(np.float32)
        sc[:, 16:24] = np.arange(c * ROWS, (c + 1) * ROWS).reshape(RT, 128).T
        sc[:, 24] = ethr
        in_maps.append({
            "xin": xin,
            "rows4": rows4,
            "scal": np.ascontiguousarray(sc),
            "selm": selm,
        })
    return x, did, iid, key, in_maps


def _assemble(did, key, outs):
    # outs: list of [128, 4, RT] per core -> per-row quantities [N, 4]
    per_core = [o.transpose(2, 0, 1).reshape(ROWS, 4) for o in outs]
    q = np.concatenate(per_core, axis=0).astype(np.float64)
    tot, pos, sdiff, shard = q[:, 0], q[:, 1], q[:, 2], q[:, 3]
    neg = tot - pos + 0.5 * sdiff + 1.5 * shard
    loss = -(np.log(pos + EPS) - np.log(pos + neg + EPS))
    counts = np.bincount(key, minlength=512)
    valid = counts[key] > 1
    n_valid = int(valid.sum())
    if n_valid > 0:
        res = float(np.sum(loss[valid]) / n_valid)
    else:
        res = float(loss.mean())
    return np.asarray(np.float32(res))


def kernel(feats, dataset_ids, image_ids, _trace=False, _ret_res=False):
    x, did, iid, key, in_maps = _prep_inputs(feats, dataset_ids, image_ids)
    nc = _get_nc()
    try:
        res = run_bass_kernel_spmd(nc, in_maps, list(range(NCORES)), trace=_trace)
    except ModuleNotFoundError:
        res = run_bass_kernel_spmd(nc, in_maps, list(range(NCORES)), trace=False)
    out = _assemble(did, key, [res.results[c]["out"] for c in range(NCORES)])
    if _ret_res:
        return out, res
    return out
